# revision 32
# baseline (speedup 1.0000x reference)
"""Trainium2 Bass kernel for GQA decode attention (nn_Attention_45844480917562).

Tensor-parallel over 8 NeuronCores: each core owns 4 query heads + 1 KV head
(wq/wk/wv column-sharded). The output projection is reduction-parallel: each
core computes its partial wo product transposed and a per-sample-group
ReduceScatter(add) leaves each core its own 512 output-feature rows; the host
only concatenates/transposes.

Compute dtype is bf16 (fp32 PSUM accumulation, fp32 softmax denominator /
division); BASS_ATTN_F32=1 switches to full fp32 at ~2x the HBM traffic.

Self-contained: hardcodes all shapes; host-side prep reshapes/transposes the
full inputs into per-core DMA-friendly layouts (K cache transposed to
[head_dim, pos], V cache chunk-major with a fused ones-column that yields the
softmax denominator for free in the P@V matmul).
"""

import os
import sys
import math

sys.path.insert(0, "/opt/trn_rl_repo")

import numpy as np
import ml_dtypes

import concourse.bass as bass
import concourse.mybir as mybir
from concourse import tile, bacc, masks
from concourse.bass_utils import run_bass_kernel_spmd

# ---------------- problem constants ----------------
DIM = 4096
N_HEADS = 32
N_KV_HEADS = 8
HEAD_DIM = 128
NCORE = 8
HPC = N_HEADS // NCORE            # 4 query heads per core
QF = HPC * HEAD_DIM               # 512 features per core
BSZ = (16, 16)
SP = (2048, 1024)                 # start_pos per group
TOT_B = 32
NFULL = (SP[0] // 128, SP[1] // 128)   # full 128-pos chunks per group: 16, 8
KCH = DIM // 128                  # 32 contraction chunks

USE_F32 = bool(int(os.environ.get("BASS_ATTN_F32", "0")))
DT = mybir.dt.float32 if USE_F32 else mybir.dt.bfloat16
NPDT = np.float32 if USE_F32 else ml_dtypes.bfloat16
SPT = 1 if USE_F32 else 4          # samples per KV tile (f32 tiles are 2x bytes)
WQ_BUFS = 2 if USE_F32 else 4

f32 = mybir.dt.float32


def _build_nc():
    nc = bacc.Bacc(trn_type="TRN2", num_devices=NCORE, enable_asserts=True)

    # ---- I/O ----
    xh = nc.dram_tensor("xh", [128, KCH, TOT_B], DT, kind="ExternalInput")
    wqkv = nc.dram_tensor("wqkv", [128, KCH, QF + 2 * HEAD_DIM], DT, kind="ExternalInput")
    # wo in [local_c, f] layout: wo_cf[p, h, f] = wo[f, 512*r + h*128 + p]
    wo = nc.dram_tensor("wo", [128, HPC, DIM], DT, kind="ExternalInput")
    kt0 = nc.dram_tensor("kt0", [BSZ[0], 128, SP[0]], DT, kind="ExternalInput")
    kt1 = nc.dram_tensor("kt1", [BSZ[1], 128, SP[1]], DT, kind="ExternalInput")
    vp0 = nc.dram_tensor("vp0", [BSZ[0], 128, NFULL[0], 129], DT, kind="ExternalInput")
    vp1 = nc.dram_tensor("vp1", [BSZ[1], 128, NFULL[1], 129], DT, kind="ExternalInput")
    ropec = nc.dram_tensor("ropec", [128, TOT_B], f32, kind="ExternalInput")
    ropes = nc.dram_tensor("ropes", [128, TOT_B], f32, kind="ExternalInput")
    # yT: rows = this core's 512 output features (f = 512*r + row), cols = samples
    y = nc.dram_tensor("y", [QF, TOT_B], f32, kind="ExternalOutput")

    WQKV_W = QF + 2 * HEAD_DIM  # 768
    SWAP_MASK = [i ^ 1 for i in range(32)]

    with tile.TileContext(nc) as tc:
        with tc.tile_pool(name="cpool", bufs=1) as cpool, \
             tc.tile_pool(name="wpool", bufs=2) as wpool, \
             tc.tile_pool(name="kvpool", bufs=2) as kvpool, \
             tc.tile_pool(name="apool", bufs=3) as apool, \
             tc.tile_pool(name="ps_t", bufs=2, space="PSUM") as ps_t, \
             tc.tile_pool(name="dpool", bufs=1, space="DRAM") as dpool:

            # ---------- constants ----------
            ident = cpool.tile([128, 128], f32)
            masks.make_identity(nc, ident[:])
            identdt = cpool.tile([TOT_B, TOT_B], DT)
            masks.make_identity(nc, identdt[:])

            # x + wqkv go at the head of the SP ring (same ring as the KV
            # stream) so the QKV critical chain gets full DMA bandwidth
            # before the bulk KV traffic.
            x_sb = cpool.tile([128, KCH * TOT_B], DT)
            nc.sync.dma_start(x_sb[:].rearrange("p (c b) -> p c b", c=KCH), xh[:])
            ropec_sb = cpool.tile([128, TOT_B], f32)
            nc.scalar.dma_start(ropec_sb[:], ropec[:])
            ropes_sb = cpool.tile([128, TOT_B], f32)
            nc.scalar.dma_start(ropes_sb[:], ropes[:])

            # ---------- phase A: QKV projection ----------
            with tc.tile_pool(name="ps_a", bufs=1, space="PSUM") as ps_a:
                qkv_ps = ps_a.tile([TOT_B, WQKV_W], f32)
                for P in range(4):
                    wq_t = wpool.tile([128, 8 * WQKV_W], DT, tag="wq", bufs=WQ_BUFS)
                    nc.sync.dma_start(
                        wq_t[:].rearrange("p (c j) -> p c j", c=8),
                        wqkv[:, 8 * P:8 * P + 8, :],
                    )
                    for ci in range(8):
                        c = 8 * P + ci
                        lhs = x_sb[:, TOT_B * c:TOT_B * (c + 1)]
                        rhs = wq_t[:, WQKV_W * ci:WQKV_W * (ci + 1)]
                        nc.tensor.matmul(qkv_ps[:, 0:512], lhs, rhs[:, 0:512],
                                         start=(c == 0), stop=(c == KCH - 1))
                        nc.tensor.matmul(qkv_ps[:, 512:768], lhs, rhs[:, 512:768],
                                         start=(c == 0), stop=(c == KCH - 1))

                qkv_sb = cpool.tile([TOT_B, WQKV_W], f32)
                nc.scalar.copy(qkv_sb[:], qkv_ps[:])

            # wo weights prefetch tile; the DMA is issued mid-stream (after
            # group 0's KV loads are queued) on the ACT ring
            wo_all = wpool.tile([128, KCH * QF], DT, tag="wo", bufs=1)

            # new-position V (plus ones column for the softmax denominator)
            vnew = cpool.tile([TOT_B, 129], DT)
            nc.vector.tensor_copy(vnew[:, 0:HEAD_DIM], qkv_sb[:, 640:768])
            nc.vector.memset(vnew[:, 128:129], 1.0)

            # ---------- transpose q heads + k, apply RoPE ----------
            qT4 = cpool.tile([128, HPC * TOT_B], DT)   # col = b*4 + h
            kTn = cpool.tile([128, TOT_B], DT)         # col = b
            for h in range(HPC + 1):                   # 4 q heads then k
                tp = ps_t.tile([128, TOT_B], f32, tag="tp")
                nc.tensor.transpose(tp[:], qkv_sb[:, 128 * h:128 * (h + 1)],
                                    ident[0:TOT_B, 0:TOT_B])
                t_sb = apool.tile([128, TOT_B], f32, tag="tr")
                nc.vector.tensor_copy(t_sb[:], tp[:])
                sw = apool.tile([128, TOT_B], f32, tag="sw")
                nc.vector.stream_shuffle(sw[:], t_sb[:], SWAP_MASK)
                t1 = apool.tile([128, TOT_B], f32, tag="t1")
                nc.vector.tensor_mul(t1[:], t_sb[:], ropec_sb[:])
                nc.vector.tensor_mul(sw[:], sw[:], ropes_sb[:])
                if h < HPC:
                    dest = qT4[:, h::HPC]
                else:
                    dest = kTn[:]
                nc.vector.tensor_add(dest, t1[:], sw[:])

            # ---------- phase B: attention over the KV cache ----------
            attnT = cpool.tile([128, HPC * TOT_B], DT)  # col = h*32 + b
            kts = (kt0, kt1)
            vps = (vp0, vp1)
            # Output projection is reduction-parallel: each core computes its
            # partial wo product (transposed, [4096, 16] per sample group) and
            # a ReduceScatter(add) sums across cores, leaving each core its
            # own 512 output-feature rows. Split per sample-group so the
            # first collective overlaps the second group's attention.
            rs_in = [dpool.tile([DIM, 16], f32, name=f"rs_in{g}") for g in range(2)]
            rs_out = [dpool.tile([QF, 16], f32, name=f"rs_out{g}")
                      for g in range(2)]
            with tc.tile_pool(name="ps_b", bufs=2, space="PSUM") as ps_b:
                for g in range(2):
                    npos = SP[g]
                    nf = NFULL[g]
                    ncol = 4 * nf
                    vw = 129 * nf
                    spt = SPT
                    for pair in range(BSZ[g] // spt):
                        ktile = kvpool.tile([128, SPT * SP[0]], DT, tag="kt")
                        nc.sync.dma_start(
                            ktile[:, 0:spt * npos].rearrange("p (s n) -> p s n", s=spt),
                            kts[g][spt * pair:spt * (pair + 1)].rearrange("s p n -> p s n"),
                        )
                        vtile = kvpool.tile([128, SPT * 129 * NFULL[0]], DT, tag="vt")
                        nc.sync.dma_start(
                            vtile[:, 0:spt * vw].rearrange("p (s c d) -> p s c d", s=spt, c=nf),
                            vps[g][spt * pair:spt * (pair + 1)].rearrange("s p c d -> p s c d"),
                        )
                        for j in range(spt):
                            b = 16 * g + spt * pair + j
                            ks = ktile[:, j * npos:(j + 1) * npos]
                            vs = vtile[:, j * vw:(j + 1) * vw]
                            q_b = qT4[:, HPC * b:HPC * (b + 1)]

                            sc_ps = ps_b.tile([128, 68], f32, tag="sc")
                            for c in range(nf):
                                nc.tensor.matmul(sc_ps[:, 4 * c:4 * c + 4],
                                                 ks[:, 128 * c:128 * (c + 1)], q_b,
                                                 start=True, stop=True)
                            nc.tensor.matmul(sc_ps[0:1, ncol:ncol + 4],
                                             kTn[:, b:b + 1], q_b,
                                             start=True, stop=True)

                            pr = apool.tile([128, 68], DT, tag="pr")
                            nc.scalar.activation(pr[:, 0:ncol], sc_ps[:, 0:ncol],
                                                 mybir.ActivationFunctionType.Exp)
                            nc.scalar.activation(pr[0:1, ncol:ncol + 4],
                                                 sc_ps[0:1, ncol:ncol + 4],
                                                 mybir.ActivationFunctionType.Exp)

                            # select row b of vnew into partition 0 (psum), for the
                            # tail matmul rhs (moving operand must be partition-0 based)
                            vrow_ps = ps_b.tile([1, 129], f32, tag="vr", bufs=1)
                            nc.tensor.matmul(vrow_ps[:], identdt[:, b:b + 1], vnew[:],
                                             start=True, stop=True)
                            vrow = apool.tile([1, 129], DT, tag="vrow")
                            nc.vector.tensor_copy(vrow[:], vrow_ps[:])

                            o_ps = ps_b.tile([HPC, 129], f32, tag="o")
                            for c in range(nf):
                                nc.tensor.matmul(o_ps[:], pr[:, 4 * c:4 * c + 4],
                                                 vs[:, 129 * c:129 * (c + 1)],
                                                 start=(c == 0), stop=False)
                            nc.tensor.matmul(o_ps[:], pr[0:1, ncol:ncol + 4],
                                             vrow[:], start=False, stop=True)

                            rec = apool.tile([HPC, 1], f32, tag="rec")
                            nc.vector.reciprocal(rec[:], o_ps[:, 128:129])
                            at = apool.tile([HPC, HEAD_DIM], f32, tag="at")
                            nc.vector.tensor_scalar_mul(at[:], o_ps[:, 0:HEAD_DIM], rec[:])

                            tp2 = ps_t.tile([128, TOT_B], f32, tag="tp")
                            nc.tensor.transpose(tp2[:, 0:HPC], at[:], ident[0:HPC, 0:HPC])
                            nc.vector.tensor_copy(attnT[:, b::TOT_B], tp2[:, 0:HPC])

                    if g == 0:
                        nc.scalar.dma_start(
                            wo_all[:].rearrange("p (c j) -> p c j", c=HPC), wo[:])

                    # this group's samples are done: partial wo product
                    # partialT[f, b] = sum_c wo[f, c] * attn[b, c]  (c = own features)
                    pT_sb = apool.tile([128, 32 * 16], f32, tag="pt", bufs=2)
                    for fq in range(8):          # 4 fb blocks per PSUM bank
                        pt_ps = ps_t.tile([128, 64], f32, tag="tp")
                        for fi in range(4):
                            fb = 4 * fq + fi
                            for h in range(HPC):
                                nc.tensor.matmul(
                                    pt_ps[:, 16 * fi:16 * (fi + 1)],
                                    wo_all[:, h * DIM + 128 * fb:h * DIM + 128 * (fb + 1)],
                                    attnT[:, TOT_B * h + 16 * g:TOT_B * h + 16 * (g + 1)],
                                    start=(h == 0), stop=(h == HPC - 1))
                        nc.vector.tensor_copy(pT_sb[:, 64 * fq:64 * (fq + 1)], pt_ps[:])
                    # rs_in row order is permuted within each rank's 512-row
                    # block (row = 512r + 4p + fb%4) so DRAM writes are 256B
                    # contiguous runs instead of 64B; host un-permutes.
                    nc.gpsimd.dma_start(
                        rs_in[g][:].rearrange("(r p four) b -> p r (four b)",
                                              r=NCORE, four=HPC),
                        pT_sb[:].rearrange("p (r four b) -> p r (four b)",
                                           r=NCORE, four=HPC),
                    )
                    nc.gpsimd.collective_compute(
                        "ReduceScatter", mybir.AluOpType.add,
                        replica_groups=[list(range(NCORE))],
                        ins=[rs_in[g].opt()], outs=[rs_out[g].opt()],
                    )
                    # y writeback on the SP ring: it waits on the collective,
                    # and SP has nothing queued behind it (ACT would stall its
                    # exp stream on this wait)
                    nc.sync.dma_start(y[:, 16 * g:16 * (g + 1)], rs_out[g][:])

    nc.finalize()
    return nc


_NC_CACHE = None


def _get_nc():
    global _NC_CACHE
    if _NC_CACHE is None:
        _NC_CACHE = _build_nc()
    return _NC_CACHE


def _prep_inputs(inputs):
    """Shard + lay out the full inputs for the 8 cores."""
    x = np.asarray(inputs["x"], np.float32)
    wq = np.asarray(inputs["wq"], np.float32)
    wk = np.asarray(inputs["wk"], np.float32)
    wv = np.asarray(inputs["wv"], np.float32)
    wo = np.asarray(inputs["wo"], np.float32)
    fc = np.asarray(inputs["freqs_cos"], np.float32)
    fs = np.asarray(inputs["freqs_sin"], np.float32)
    caches = (
        (np.asarray(inputs["cache_k0"], np.float32), np.asarray(inputs["cache_v0"], np.float32)),
        (np.asarray(inputs["cache_k1"], np.float32), np.asarray(inputs["cache_v1"], np.float32)),
    )

    x_flat = x.reshape(TOT_B, DIM)
    xh = np.ascontiguousarray(
        x_flat.T.reshape(KCH, 128, TOT_B).transpose(1, 0, 2)
    ).astype(NPDT)

    # RoPE tables: per-column position (2048 for tokens 0-15, 1024 for 16-31)
    C = np.empty((128, TOT_B), np.float32)
    S = np.empty((128, TOT_B), np.float32)
    for g in range(2):
        cos = fc[SP[g]]
        sin = fs[SP[g]]
        cols = slice(16 * g, 16 * (g + 1))
        C[0::2, cols] = cos[:, None]
        C[1::2, cols] = cos[:, None]
        S[0::2, cols] = -sin[:, None]
        S[1::2, cols] = sin[:, None]

    scale = 1.0 / math.sqrt(HEAD_DIM)
    in_maps = []
    for r in range(NCORE):
        w_q = wq[QF * r:QF * (r + 1)] * scale
        w_k = wk[HEAD_DIM * r:HEAD_DIM * (r + 1)]
        w_v = wv[HEAD_DIM * r:HEAD_DIM * (r + 1)]
        wqkvT = np.concatenate([w_q, w_k, w_v], axis=0).T  # [4096, 768]
        wqkv_hp = np.ascontiguousarray(
            wqkvT.reshape(KCH, 128, 768).transpose(1, 0, 2)
        ).astype(NPDT)

        # wo_cf[local_c, f] = wo[f, 512r + local_c]  -> [128, HPC, 4096]
        wo_cf = wo[:, QF * r:QF * (r + 1)].T  # [512, 4096]
        wo_hp = np.ascontiguousarray(
            wo_cf.reshape(HPC, 128, DIM).transpose(1, 0, 2)
        ).astype(NPDT)

        m = {"xh": xh, "wqkv": wqkv_hp, "wo": wo_hp,
             "ropec": C, "ropes": S}
        for g in range(2):
            ck, cv = caches[g]
            npos = SP[g]
            nf = NFULL[g]
            # cast to the wire dtype first, then do the layout copy at half width
            kslab = ck[:, :npos, r, :].astype(NPDT)       # [16, npos, 128]
            kt = np.ascontiguousarray(kslab.transpose(0, 2, 1))  # [16, 128, npos]
            vslab = cv[:, :npos, r, :].astype(NPDT).reshape(BSZ[g], nf, 128, HEAD_DIM)
            vp = np.empty((BSZ[g], 128, nf, 129), NPDT)
            vp[:, :, :, HEAD_DIM] = NPDT(1.0)
            vp[:, :, :, :HEAD_DIM] = vslab.transpose(0, 2, 1, 3)
            m[f"kt{g}"] = kt
            m[f"vp{g}"] = vp
        in_maps.append(m)
    return in_maps


def _run(inputs, trace=False):
    nc = _get_nc()
    in_maps = _prep_inputs(inputs)
    res = run_bass_kernel_spmd(nc, in_maps, core_ids=list(range(NCORE)), trace=trace)
    # each core returns yT rows [512r : 512r+512] of the [4096, 32] output,
    # row-permuted within the block (row = 4p + fb%4 -> f_local = 128*(fb%4) + p)
    parts = []
    for r in range(NCORE):
        yr = res.results[r]["y"]  # [512, 32]
        parts.append(yr.reshape(128, HPC, TOT_B).transpose(1, 0, 2).reshape(QF, TOT_B))
    y_t = np.concatenate(parts, axis=0)
    out = np.ascontiguousarray(y_t.T).reshape(TOT_B, 1, DIM).astype(np.float32)
    return out, res


def kernel(**inputs):
    out, _ = _run(inputs, trace=False)
    return out


# revision 42
# speedup vs baseline: 1.9275x; 1.9275x over previous
"""Trainium2 Bass kernel for GQA decode attention (nn_Attention_45844480917562).

Tensor-parallel over 8 NeuronCores: each core owns 4 query heads + 1 KV head
(wq/wk/wv column-sharded). The output projection is reduction-parallel: each
core computes its partial wo product transposed and a per-sample-group
ReduceScatter(add) leaves each core its own 512 output-feature rows; the host
only concatenates/transposes.

Compute dtype is bf16 (fp32 PSUM accumulation, fp32 softmax denominator /
division); BASS_ATTN_F32=1 switches to full fp32 at ~2x the HBM traffic.

Self-contained: hardcodes all shapes; host-side prep reshapes/transposes the
full inputs into per-core DMA-friendly layouts (K cache transposed to
[head_dim, pos], V cache chunk-major with a fused ones-column that yields the
softmax denominator for free in the P@V matmul).
"""

import os
import sys
import math

sys.path.insert(0, "/opt/trn_rl_repo")

import numpy as np
import ml_dtypes

import concourse.bass as bass
import concourse.mybir as mybir
from concourse import tile, bacc, masks
from concourse.bass_utils import run_bass_kernel_spmd

# ---------------- problem constants ----------------
DIM = 4096
N_HEADS = 32
N_KV_HEADS = 8
HEAD_DIM = 128
NCORE = 8
HPC = N_HEADS // NCORE            # 4 query heads per core
QF = HPC * HEAD_DIM               # 512 features per core
BSZ = (16, 16)
SP = (2048, 1024)                 # start_pos per group
TOT_B = 32
NFULL = (SP[0] // 128, SP[1] // 128)   # full 128-pos chunks per group: 16, 8
KCH = DIM // 128                  # 32 contraction chunks

USE_F32 = bool(int(os.environ.get("BASS_ATTN_F32", "0")))
DT = mybir.dt.float32 if USE_F32 else mybir.dt.bfloat16
NPDT = np.float32 if USE_F32 else ml_dtypes.bfloat16
SPT = 1 if USE_F32 else 4          # samples per KV tile (f32 tiles are 2x bytes)
WQ_BUFS = 2 if USE_F32 else 4

f32 = mybir.dt.float32


def _build_nc():
    nc = bacc.Bacc(trn_type="TRN2", num_devices=NCORE, enable_asserts=True)

    # ---- I/O ----
    xh = nc.dram_tensor("xh", [128, KCH, TOT_B], DT, kind="ExternalInput")
    wqkv = nc.dram_tensor("wqkv", [128, KCH, QF + 2 * HEAD_DIM], DT, kind="ExternalInput")
    # wo in [local_c, f] layout: wo_cf[p, h, f] = wo[f, 512*r + h*128 + p]
    wo = nc.dram_tensor("wo", [128, HPC, DIM], DT, kind="ExternalInput")
    kt0 = nc.dram_tensor("kt0", [BSZ[0], 128, SP[0]], DT, kind="ExternalInput")
    kt1 = nc.dram_tensor("kt1", [BSZ[1], 128, SP[1]], DT, kind="ExternalInput")
    vp0 = nc.dram_tensor("vp0", [BSZ[0], 128, NFULL[0], 129], DT, kind="ExternalInput")
    vp1 = nc.dram_tensor("vp1", [BSZ[1], 128, NFULL[1], 129], DT, kind="ExternalInput")
    ropec = nc.dram_tensor("ropec", [128, TOT_B], f32, kind="ExternalInput")
    ropes = nc.dram_tensor("ropes", [128, TOT_B], f32, kind="ExternalInput")
    # yT: rows = this core's 512 output features (f = 512*r + row), cols = samples
    y = nc.dram_tensor("y", [QF, TOT_B], f32, kind="ExternalOutput")

    WQKV_W = QF + 2 * HEAD_DIM  # 768
    SWAP_MASK = [i ^ 1 for i in range(32)]

    with tile.TileContext(nc) as tc:
        with tc.tile_pool(name="cpool", bufs=1) as cpool, \
             tc.tile_pool(name="wpool", bufs=2) as wpool, \
             tc.tile_pool(name="kvpool", bufs=2) as kvpool, \
             tc.tile_pool(name="apool", bufs=3) as apool, \
             tc.tile_pool(name="ps_t", bufs=2, space="PSUM") as ps_t, \
             tc.tile_pool(name="dpool", bufs=1, space="DRAM") as dpool:

            # ---------- constants ----------
            ident = cpool.tile([128, 128], f32)
            masks.make_identity(nc, ident[:])
            identdt = cpool.tile([TOT_B, TOT_B], DT)
            masks.make_identity(nc, identdt[:])

            # x + wqkv go at the head of the SP ring (same ring as the KV
            # stream) so the QKV critical chain gets full DMA bandwidth
            # before the bulk KV traffic.
            x_sb = cpool.tile([128, KCH * TOT_B], DT)
            nc.sync.dma_start(x_sb[:].rearrange("p (c b) -> p c b", c=KCH), xh[:])
            ropec_sb = cpool.tile([128, TOT_B], f32)
            nc.scalar.dma_start(ropec_sb[:], ropec[:])
            ropes_sb = cpool.tile([128, TOT_B], f32)
            nc.scalar.dma_start(ropes_sb[:], ropes[:])

            # ---------- phase A: QKV projection ----------
            with tc.tile_pool(name="ps_a", bufs=1, space="PSUM") as ps_a:
                qkv_ps = ps_a.tile([TOT_B, WQKV_W], f32)
                for P in range(4):
                    wq_t = wpool.tile([128, 8 * WQKV_W], DT, tag="wq", bufs=WQ_BUFS)
                    nc.sync.dma_start(
                        wq_t[:].rearrange("p (c j) -> p c j", c=8),
                        wqkv[:, 8 * P:8 * P + 8, :],
                    )
                    for ci in range(8):
                        c = 8 * P + ci
                        lhs = x_sb[:, TOT_B * c:TOT_B * (c + 1)]
                        rhs = wq_t[:, WQKV_W * ci:WQKV_W * (ci + 1)]
                        nc.tensor.matmul(qkv_ps[:, 0:512], lhs, rhs[:, 0:512],
                                         start=(c == 0), stop=(c == KCH - 1))
                        nc.tensor.matmul(qkv_ps[:, 512:768], lhs, rhs[:, 512:768],
                                         start=(c == 0), stop=(c == KCH - 1))

                qkv_sb = cpool.tile([TOT_B, WQKV_W], f32)
                nc.scalar.copy(qkv_sb[:], qkv_ps[:])

            # wo weights prefetch tile; the DMA is issued mid-stream (after
            # group 0's KV loads are queued) on the ACT ring
            wo_all = wpool.tile([128, KCH * QF], DT, tag="wo", bufs=1)

            # new-position V (plus ones column for the softmax denominator)
            vnew = cpool.tile([TOT_B, 129], DT)
            nc.vector.tensor_copy(vnew[:, 0:HEAD_DIM], qkv_sb[:, 640:768])
            nc.vector.memset(vnew[:, 128:129], 1.0)

            # ---------- transpose q heads + k, apply RoPE ----------
            qT4 = cpool.tile([128, HPC * TOT_B], DT)   # col = b*4 + h
            kTn = cpool.tile([128, TOT_B], DT)         # col = b
            for h in range(HPC + 1):                   # 4 q heads then k
                tp = ps_t.tile([128, TOT_B], f32, tag="tp")
                nc.tensor.transpose(tp[:], qkv_sb[:, 128 * h:128 * (h + 1)],
                                    ident[0:TOT_B, 0:TOT_B])
                t_sb = apool.tile([128, TOT_B], f32, tag="tr")
                nc.vector.tensor_copy(t_sb[:], tp[:])
                sw = apool.tile([128, TOT_B], f32, tag="sw")
                nc.vector.stream_shuffle(sw[:], t_sb[:], SWAP_MASK)
                t1 = apool.tile([128, TOT_B], f32, tag="t1")
                nc.vector.tensor_mul(t1[:], t_sb[:], ropec_sb[:])
                nc.vector.tensor_mul(sw[:], sw[:], ropes_sb[:])
                if h < HPC:
                    dest = qT4[:, h::HPC]
                else:
                    dest = kTn[:]
                nc.vector.tensor_add(dest, t1[:], sw[:])

            # ---------- phase B: attention over the KV cache ----------
            attnT = cpool.tile([128, HPC * TOT_B], DT)  # col = h*32 + b
            kts = (kt0, kt1)
            vps = (vp0, vp1)
            # Output projection is reduction-parallel: each core computes its
            # partial wo product (transposed, [4096, 16] per sample group) and
            # a ReduceScatter(add) sums across cores, leaving each core its
            # own 512 output-feature rows. Split per sample-group so the
            # first collective overlaps the second group's attention.
            rs_in = [dpool.tile([DIM, 16], f32, name=f"rs_in{g}") for g in range(2)]
            rs_out = [dpool.tile([QF, 16], f32, name=f"rs_out{g}")
                      for g in range(2)]
            with tc.tile_pool(name="ps_b", bufs=2, space="PSUM") as ps_b:
                for g in range(2):
                    npos = SP[g]
                    nf = NFULL[g]
                    ncol = 4 * nf
                    vw = 129 * nf
                    spt = SPT
                    for pair in range(BSZ[g] // spt):
                        ktile = kvpool.tile([128, SPT * SP[0]], DT, tag="kt")
                        nc.sync.dma_start(
                            ktile[:, 0:spt * npos].rearrange("p (s n) -> p s n", s=spt),
                            kts[g][spt * pair:spt * (pair + 1)].rearrange("s p n -> p s n"),
                        )
                        vtile = kvpool.tile([128, SPT * 129 * NFULL[0]], DT, tag="vt")
                        nc.sync.dma_start(
                            vtile[:, 0:spt * vw].rearrange("p (s c d) -> p s c d", s=spt, c=nf),
                            vps[g][spt * pair:spt * (pair + 1)].rearrange("s p c d -> p s c d"),
                        )
                        for j in range(spt):
                            b = 16 * g + spt * pair + j
                            ks = ktile[:, j * npos:(j + 1) * npos]
                            vs = vtile[:, j * vw:(j + 1) * vw]
                            q_b = qT4[:, HPC * b:HPC * (b + 1)]

                            sc_ps = ps_b.tile([128, 68], f32, tag="sc")
                            for c in range(nf):
                                nc.tensor.matmul(sc_ps[:, 4 * c:4 * c + 4],
                                                 ks[:, 128 * c:128 * (c + 1)], q_b,
                                                 start=True, stop=True)
                            nc.tensor.matmul(sc_ps[0:1, ncol:ncol + 4],
                                             kTn[:, b:b + 1], q_b,
                                             start=True, stop=True)

                            pr = apool.tile([128, 68], DT, tag="pr")
                            nc.scalar.activation(pr[:, 0:ncol], sc_ps[:, 0:ncol],
                                                 mybir.ActivationFunctionType.Exp)
                            nc.scalar.activation(pr[0:1, ncol:ncol + 4],
                                                 sc_ps[0:1, ncol:ncol + 4],
                                                 mybir.ActivationFunctionType.Exp)

                            # select row b of vnew into partition 0 (psum), for the
                            # tail matmul rhs (moving operand must be partition-0 based)
                            vrow_ps = ps_b.tile([1, 129], f32, tag="vr", bufs=1)
                            nc.tensor.matmul(vrow_ps[:], identdt[:, b:b + 1], vnew[:],
                                             start=True, stop=True)
                            vrow = apool.tile([1, 129], DT, tag="vrow")
                            nc.vector.tensor_copy(vrow[:], vrow_ps[:])

                            o_ps = ps_b.tile([HPC, 129], f32, tag="o")
                            for c in range(nf):
                                nc.tensor.matmul(o_ps[:], pr[:, 4 * c:4 * c + 4],
                                                 vs[:, 129 * c:129 * (c + 1)],
                                                 start=(c == 0), stop=False)
                            nc.tensor.matmul(o_ps[:], pr[0:1, ncol:ncol + 4],
                                             vrow[:], start=False, stop=True)

                            rec = apool.tile([HPC, 1], f32, tag="rec")
                            nc.vector.reciprocal(rec[:], o_ps[:, 128:129])
                            at = apool.tile([HPC, HEAD_DIM], f32, tag="at")
                            nc.vector.tensor_scalar_mul(at[:], o_ps[:, 0:HEAD_DIM], rec[:])

                            tp2 = ps_t.tile([128, TOT_B], f32, tag="tp")
                            nc.tensor.transpose(tp2[:, 0:HPC], at[:], ident[0:HPC, 0:HPC])
                            nc.vector.tensor_copy(attnT[:, b::TOT_B], tp2[:, 0:HPC])

                    if g == 0:
                        nc.scalar.dma_start(
                            wo_all[:].rearrange("p (c j) -> p c j", c=HPC), wo[:])

                    # this group's samples are done: partial wo product
                    # partialT[f, b] = sum_c wo[f, c] * attn[b, c]  (c = own features)
                    pT_sb = apool.tile([128, 32 * 16], f32, tag="pt", bufs=2)
                    for fq in range(8):          # 4 fb blocks per PSUM bank
                        pt_ps = ps_t.tile([128, 64], f32, tag="tp")
                        for fi in range(4):
                            fb = 4 * fq + fi
                            for h in range(HPC):
                                nc.tensor.matmul(
                                    pt_ps[:, 16 * fi:16 * (fi + 1)],
                                    wo_all[:, h * DIM + 128 * fb:h * DIM + 128 * (fb + 1)],
                                    attnT[:, TOT_B * h + 16 * g:TOT_B * h + 16 * (g + 1)],
                                    start=(h == 0), stop=(h == HPC - 1))
                        nc.vector.tensor_copy(pT_sb[:, 64 * fq:64 * (fq + 1)], pt_ps[:])
                        # rs_in row order is permuted within each rank's
                        # 512-row block (row = 512r + 4p + fb%4) so DRAM
                        # writes are 256B contiguous runs instead of 64B;
                        # host un-permutes. Shipped in two rank-halves so the
                        # first overlaps the remaining partial matmuls.
                        if fq == 3 or fq == 7:
                            half = fq // 4
                            nc.gpsimd.dma_start(
                                rs_in[g][2048 * half:2048 * (half + 1)].rearrange(
                                    "(r p four) b -> p r (four b)", r=NCORE // 2, four=HPC),
                                pT_sb[:, 256 * half:256 * (half + 1)].rearrange(
                                    "p (r four b) -> p r (four b)", r=NCORE // 2, four=HPC),
                            )
                    nc.gpsimd.collective_compute(
                        "ReduceScatter", mybir.AluOpType.add,
                        replica_groups=[list(range(NCORE))],
                        ins=[rs_in[g].opt()], outs=[rs_out[g].opt()],
                    )
                    # y writeback on the SP ring: it waits on the collective,
                    # and SP has nothing queued behind it (ACT would stall its
                    # exp stream on this wait)
                    nc.sync.dma_start(y[:, 16 * g:16 * (g + 1)], rs_out[g][:])

    nc.finalize()
    return nc


_NC_CACHE = None


def _get_nc():
    global _NC_CACHE
    if _NC_CACHE is None:
        _NC_CACHE = _build_nc()
    return _NC_CACHE


def _prep_inputs(inputs):
    """Shard + lay out the full inputs for the 8 cores."""
    x = np.asarray(inputs["x"], np.float32)
    wq = np.asarray(inputs["wq"], np.float32)
    wk = np.asarray(inputs["wk"], np.float32)
    wv = np.asarray(inputs["wv"], np.float32)
    wo = np.asarray(inputs["wo"], np.float32)
    fc = np.asarray(inputs["freqs_cos"], np.float32)
    fs = np.asarray(inputs["freqs_sin"], np.float32)
    caches = (
        (np.asarray(inputs["cache_k0"], np.float32), np.asarray(inputs["cache_v0"], np.float32)),
        (np.asarray(inputs["cache_k1"], np.float32), np.asarray(inputs["cache_v1"], np.float32)),
    )

    x_flat = x.reshape(TOT_B, DIM)
    xh = np.ascontiguousarray(
        x_flat.T.reshape(KCH, 128, TOT_B).transpose(1, 0, 2)
    ).astype(NPDT)

    # RoPE tables: per-column position (2048 for tokens 0-15, 1024 for 16-31)
    C = np.empty((128, TOT_B), np.float32)
    S = np.empty((128, TOT_B), np.float32)
    for g in range(2):
        cos = fc[SP[g]]
        sin = fs[SP[g]]
        cols = slice(16 * g, 16 * (g + 1))
        C[0::2, cols] = cos[:, None]
        C[1::2, cols] = cos[:, None]
        S[0::2, cols] = -sin[:, None]
        S[1::2, cols] = sin[:, None]

    scale = 1.0 / math.sqrt(HEAD_DIM)
    in_maps = []
    for r in range(NCORE):
        w_q = wq[QF * r:QF * (r + 1)] * scale
        w_k = wk[HEAD_DIM * r:HEAD_DIM * (r + 1)]
        w_v = wv[HEAD_DIM * r:HEAD_DIM * (r + 1)]
        wqkvT = np.concatenate([w_q, w_k, w_v], axis=0).T  # [4096, 768]
        wqkv_hp = np.ascontiguousarray(
            wqkvT.reshape(KCH, 128, 768).transpose(1, 0, 2)
        ).astype(NPDT)

        # wo_cf[local_c, f] = wo[f, 512r + local_c]  -> [128, HPC, 4096]
        wo_cf = wo[:, QF * r:QF * (r + 1)].T  # [512, 4096]
        wo_hp = np.ascontiguousarray(
            wo_cf.reshape(HPC, 128, DIM).transpose(1, 0, 2)
        ).astype(NPDT)

        m = {"xh": xh, "wqkv": wqkv_hp, "wo": wo_hp,
             "ropec": C, "ropes": S}
        for g in range(2):
            ck, cv = caches[g]
            npos = SP[g]
            nf = NFULL[g]
            # cast to the wire dtype first, then do the layout copy at half width
            kslab = ck[:, :npos, r, :].astype(NPDT)       # [16, npos, 128]
            kt = np.ascontiguousarray(kslab.transpose(0, 2, 1))  # [16, 128, npos]
            vslab = cv[:, :npos, r, :].astype(NPDT).reshape(BSZ[g], nf, 128, HEAD_DIM)
            vp = np.empty((BSZ[g], 128, nf, 129), NPDT)
            vp[:, :, :, HEAD_DIM] = NPDT(1.0)
            vp[:, :, :, :HEAD_DIM] = vslab.transpose(0, 2, 1, 3)
            m[f"kt{g}"] = kt
            m[f"vp{g}"] = vp
        in_maps.append(m)
    return in_maps


def _run(inputs, trace=False):
    nc = _get_nc()
    in_maps = _prep_inputs(inputs)
    res = run_bass_kernel_spmd(nc, in_maps, core_ids=list(range(NCORE)), trace=trace)
    # each core returns yT rows [512r : 512r+512] of the [4096, 32] output,
    # row-permuted within the block (row = 4p + fb%4 -> f_local = 128*(fb%4) + p)
    parts = []
    for r in range(NCORE):
        yr = res.results[r]["y"]  # [512, 32]
        parts.append(yr.reshape(128, HPC, TOT_B).transpose(1, 0, 2).reshape(QF, TOT_B))
    y_t = np.concatenate(parts, axis=0)
    out = np.ascontiguousarray(y_t.T).reshape(TOT_B, 1, DIM).astype(np.float32)
    return out, res


def kernel(**inputs):
    out, _ = _run(inputs, trace=False)
    return out


# revision 46
# speedup vs baseline: 1.9337x; 1.0032x over previous
"""Trainium2 Bass kernel for GQA decode attention (nn_Attention_45844480917562).

Tensor-parallel over 8 NeuronCores: each core owns 4 query heads + 1 KV head
(wq/wk/wv column-sharded). The output projection is reduction-parallel: each
core computes its partial wo product transposed and a per-sample-group
ReduceScatter(add) leaves each core its own 512 output-feature rows; the host
only concatenates/transposes.

Compute dtype is bf16 (fp32 PSUM accumulation, fp32 softmax denominator /
division); BASS_ATTN_F32=1 switches to full fp32 at ~2x the HBM traffic.

Self-contained: hardcodes all shapes; host-side prep reshapes/transposes the
full inputs into per-core DMA-friendly layouts (K cache transposed to
[head_dim, pos], V cache chunk-major with a fused ones-column that yields the
softmax denominator for free in the P@V matmul).
"""

import os
import sys
import math

sys.path.insert(0, "/opt/trn_rl_repo")

import numpy as np
import ml_dtypes

import concourse.bass as bass
import concourse.mybir as mybir
from concourse import tile, bacc, masks
from concourse.bass_utils import run_bass_kernel_spmd

# ---------------- problem constants ----------------
DIM = 4096
N_HEADS = 32
N_KV_HEADS = 8
HEAD_DIM = 128
NCORE = 8
HPC = N_HEADS // NCORE            # 4 query heads per core
QF = HPC * HEAD_DIM               # 512 features per core
BSZ = (16, 16)
SP = (2048, 1024)                 # start_pos per group
TOT_B = 32
NFULL = (SP[0] // 128, SP[1] // 128)   # full 128-pos chunks per group: 16, 8
KCH = DIM // 128                  # 32 contraction chunks

USE_F32 = bool(int(os.environ.get("BASS_ATTN_F32", "0")))
DT = mybir.dt.float32 if USE_F32 else mybir.dt.bfloat16
NPDT = np.float32 if USE_F32 else ml_dtypes.bfloat16
SPT = 1 if USE_F32 else 4          # samples per KV tile (f32 tiles are 2x bytes)
WQ_BUFS = 2 if USE_F32 else 4

f32 = mybir.dt.float32


def _build_nc():
    nc = bacc.Bacc(trn_type="TRN2", num_devices=NCORE, enable_asserts=True)

    # ---- I/O ----
    xh = nc.dram_tensor("xh", [128, KCH, TOT_B], DT, kind="ExternalInput")
    wqkv = nc.dram_tensor("wqkv", [128, KCH, QF + 2 * HEAD_DIM], DT, kind="ExternalInput")
    # wo in [local_c, f] layout: wo_cf[p, h, f] = wo[f, 512*r + h*128 + p]
    wo = nc.dram_tensor("wo", [128, HPC, DIM], DT, kind="ExternalInput")
    kt0 = nc.dram_tensor("kt0", [BSZ[0], 128, SP[0]], DT, kind="ExternalInput")
    kt1 = nc.dram_tensor("kt1", [BSZ[1], 128, SP[1]], DT, kind="ExternalInput")
    vp0 = nc.dram_tensor("vp0", [BSZ[0], 128, NFULL[0], 129], DT, kind="ExternalInput")
    vp1 = nc.dram_tensor("vp1", [BSZ[1], 128, NFULL[1], 129], DT, kind="ExternalInput")
    ropec = nc.dram_tensor("ropec", [128, TOT_B], f32, kind="ExternalInput")
    ropes = nc.dram_tensor("ropes", [128, TOT_B], f32, kind="ExternalInput")
    # yT: rows = this core's 512 output features (f = 512*r + row), cols = samples
    y = nc.dram_tensor("y", [QF, TOT_B], f32, kind="ExternalOutput")

    WQKV_W = QF + 2 * HEAD_DIM  # 768
    SWAP_MASK = [i ^ 1 for i in range(32)]

    with tile.TileContext(nc) as tc:
        with tc.tile_pool(name="cpool", bufs=1) as cpool, \
             tc.tile_pool(name="wpool", bufs=2) as wpool, \
             tc.tile_pool(name="kvpool", bufs=2) as kvpool, \
             tc.tile_pool(name="apool", bufs=3) as apool, \
             tc.tile_pool(name="ps_t", bufs=2, space="PSUM") as ps_t, \
             tc.tile_pool(name="dpool", bufs=1, space="DRAM") as dpool:

            # ---------- constants ----------
            ident = cpool.tile([128, 128], f32)
            masks.make_identity(nc, ident[:])
            identdt = cpool.tile([TOT_B, TOT_B], DT)
            masks.make_identity(nc, identdt[:])

            # x + wqkv go at the head of the SP ring (same ring as the KV
            # stream) so the QKV critical chain gets full DMA bandwidth
            # before the bulk KV traffic.
            x_sb = cpool.tile([128, KCH * TOT_B], DT)
            nc.sync.dma_start(x_sb[:].rearrange("p (c b) -> p c b", c=KCH), xh[:])
            ropec_sb = cpool.tile([128, TOT_B], f32)
            nc.scalar.dma_start(ropec_sb[:], ropec[:])
            ropes_sb = cpool.tile([128, TOT_B], f32)
            nc.scalar.dma_start(ropes_sb[:], ropes[:])

            # ---------- phase A: QKV projection ----------
            with tc.tile_pool(name="ps_a", bufs=1, space="PSUM") as ps_a:
                qkv_ps = ps_a.tile([TOT_B, WQKV_W], f32)
                for P in range(4):
                    wq_t = wpool.tile([128, 8 * WQKV_W], DT, tag="wq", bufs=WQ_BUFS)
                    nc.sync.dma_start(
                        wq_t[:].rearrange("p (c j) -> p c j", c=8),
                        wqkv[:, 8 * P:8 * P + 8, :],
                    )
                    for ci in range(8):
                        c = 8 * P + ci
                        lhs = x_sb[:, TOT_B * c:TOT_B * (c + 1)]
                        rhs = wq_t[:, WQKV_W * ci:WQKV_W * (ci + 1)]
                        nc.tensor.matmul(qkv_ps[:, 0:512], lhs, rhs[:, 0:512],
                                         start=(c == 0), stop=(c == KCH - 1))
                        nc.tensor.matmul(qkv_ps[:, 512:768], lhs, rhs[:, 512:768],
                                         start=(c == 0), stop=(c == KCH - 1))

                qkv_sb = cpool.tile([TOT_B, WQKV_W], f32)
                nc.scalar.copy(qkv_sb[:], qkv_ps[:])

            # wo weights prefetch tile; the DMA is issued mid-stream (after
            # group 0's KV loads are queued) on the ACT ring
            wo_all = wpool.tile([128, KCH * QF], DT, tag="wo", bufs=1)

            # new-position V (plus ones column for the softmax denominator)
            vnew = cpool.tile([TOT_B, 129], DT)
            nc.vector.tensor_copy(vnew[:, 0:HEAD_DIM], qkv_sb[:, 640:768])
            nc.vector.memset(vnew[:, 128:129], 1.0)

            # ---------- transpose q heads + k, apply RoPE ----------
            qT4 = cpool.tile([128, HPC * TOT_B], DT)   # col = b*4 + h
            kTn = cpool.tile([128, TOT_B], DT)         # col = b
            for h in range(HPC + 1):                   # 4 q heads then k
                tp = ps_t.tile([128, TOT_B], f32, tag="tp")
                nc.tensor.transpose(tp[:], qkv_sb[:, 128 * h:128 * (h + 1)],
                                    ident[0:TOT_B, 0:TOT_B])
                t_sb = apool.tile([128, TOT_B], f32, tag="tr")
                nc.vector.tensor_copy(t_sb[:], tp[:])
                sw = apool.tile([128, TOT_B], f32, tag="sw")
                nc.vector.stream_shuffle(sw[:], t_sb[:], SWAP_MASK)
                t1 = apool.tile([128, TOT_B], f32, tag="t1")
                nc.vector.tensor_mul(t1[:], t_sb[:], ropec_sb[:])
                nc.vector.tensor_mul(sw[:], sw[:], ropes_sb[:])
                if h < HPC:
                    dest = qT4[:, h::HPC]
                else:
                    dest = kTn[:]
                nc.vector.tensor_add(dest, t1[:], sw[:])

            # ---------- phase B: attention over the KV cache ----------
            attnT = cpool.tile([128, HPC * TOT_B], DT)  # col = h*32 + b
            kts = (kt0, kt1)
            vps = (vp0, vp1)
            # Output projection is reduction-parallel: each core computes its
            # partial wo product (transposed, [4096, 16] per sample group) and
            # a ReduceScatter(add) sums across cores, leaving each core its
            # own 512 output-feature rows. Split per sample-group so the
            # first collective overlaps the second group's attention.
            rs_in = [dpool.tile([DIM, 16], f32, name=f"rs_in{g}") for g in range(2)]
            rs_out = [dpool.tile([QF, 16], f32, name=f"rs_out{g}")
                      for g in range(2)]
            with tc.tile_pool(name="ps_b", bufs=2, space="PSUM") as ps_b:
                for g in range(2):
                    npos = SP[g]
                    nf = NFULL[g]
                    ncol = 4 * nf
                    vw = 129 * nf
                    spt = SPT
                    for pair in range(BSZ[g] // spt):
                        ktile = kvpool.tile([128, SPT * SP[0]], DT, tag="kt")
                        nc.sync.dma_start(
                            ktile[:, 0:spt * npos].rearrange("p (s n) -> p s n", s=spt),
                            kts[g][spt * pair:spt * (pair + 1)].rearrange("s p n -> p s n"),
                        )
                        vtile = kvpool.tile([128, SPT * 129 * NFULL[0]], DT, tag="vt")
                        nc.sync.dma_start(
                            vtile[:, 0:spt * vw].rearrange("p (s c d) -> p s c d", s=spt, c=nf),
                            vps[g][spt * pair:spt * (pair + 1)].rearrange("s p c d -> p s c d"),
                        )
                        for j in range(spt):
                            b = 16 * g + spt * pair + j
                            ks = ktile[:, j * npos:(j + 1) * npos]
                            vs = vtile[:, j * vw:(j + 1) * vw]
                            q_b = qT4[:, HPC * b:HPC * (b + 1)]

                            sc_ps = ps_b.tile([128, 68], f32, tag="sc")
                            for c in range(nf):
                                nc.tensor.matmul(sc_ps[:, 4 * c:4 * c + 4],
                                                 ks[:, 128 * c:128 * (c + 1)], q_b,
                                                 start=True, stop=True)
                            nc.tensor.matmul(sc_ps[0:1, ncol:ncol + 4],
                                             kTn[:, b:b + 1], q_b,
                                             start=True, stop=True)

                            pr = apool.tile([128, 68], DT, tag="pr")
                            nc.scalar.activation(pr[:, 0:ncol], sc_ps[:, 0:ncol],
                                                 mybir.ActivationFunctionType.Exp)
                            nc.scalar.activation(pr[0:1, ncol:ncol + 4],
                                                 sc_ps[0:1, ncol:ncol + 4],
                                                 mybir.ActivationFunctionType.Exp)

                            # select row b of vnew into partition 0 (psum), for the
                            # tail matmul rhs (moving operand must be partition-0 based)
                            vrow_ps = ps_b.tile([1, 129], f32, tag="vr", bufs=1)
                            nc.tensor.matmul(vrow_ps[:], identdt[:, b:b + 1], vnew[:],
                                             start=True, stop=True)
                            vrow = apool.tile([1, 129], DT, tag="vrow")
                            nc.vector.tensor_copy(vrow[:], vrow_ps[:])

                            o_ps = ps_b.tile([HPC, 129], f32, tag="o")
                            for c in range(nf):
                                nc.tensor.matmul(o_ps[:], pr[:, 4 * c:4 * c + 4],
                                                 vs[:, 129 * c:129 * (c + 1)],
                                                 start=(c == 0), stop=False)
                            nc.tensor.matmul(o_ps[:], pr[0:1, ncol:ncol + 4],
                                             vrow[:], start=False, stop=True)

                            rec = apool.tile([HPC, 1], f32, tag="rec")
                            nc.vector.reciprocal(rec[:], o_ps[:, 128:129])
                            at = apool.tile([HPC, HEAD_DIM], f32, tag="at")
                            nc.vector.tensor_scalar_mul(at[:], o_ps[:, 0:HEAD_DIM], rec[:])

                            tp2 = ps_t.tile([128, TOT_B], f32, tag="tp")
                            nc.tensor.transpose(tp2[:, 0:HPC], at[:], ident[0:HPC, 0:HPC])
                            nc.vector.tensor_copy(attnT[:, b::TOT_B], tp2[:, 0:HPC])

                    if g == 0:
                        nc.scalar.dma_start(
                            wo_all[:].rearrange("p (c j) -> p c j", c=HPC), wo[:])

                    # this group's samples are done: partial wo product
                    # partialT[f, b] = sum_c wo[f, c] * attn[b, c]  (c = own features)
                    pT_sb = apool.tile([128, 32 * 16], f32, tag="pt", bufs=2)
                    for fq in range(8):          # 4 fb blocks per PSUM bank
                        pt_ps = ps_t.tile([128, 64], f32, tag="tp")
                        for fi in range(4):
                            fb = 4 * fq + fi
                            for h in range(HPC):
                                nc.tensor.matmul(
                                    pt_ps[:, 16 * fi:16 * (fi + 1)],
                                    wo_all[:, h * DIM + 128 * fb:h * DIM + 128 * (fb + 1)],
                                    attnT[:, TOT_B * h + 16 * g:TOT_B * h + 16 * (g + 1)],
                                    start=(h == 0), stop=(h == HPC - 1))
                        nc.vector.tensor_copy(pT_sb[:, 64 * fq:64 * (fq + 1)], pt_ps[:])
                        # rs_in row order is permuted within each rank's
                        # 512-row block (row = 512r + 4p + fb%4) so DRAM
                        # writes are 256B contiguous runs instead of 64B;
                        # host un-permutes. Shipped in two rank-halves so the
                        # first overlaps the remaining partial matmuls.
                        if fq == 3 or fq == 7:
                            half = fq // 4
                            # group 1's rs_in is tail-critical: use the idle
                            # ACT HWDGE ring (faster first-byte than SWDGE);
                            # group 0's stays on gpsimd mid-stream.
                            dma_eng = nc.scalar if g == 1 else nc.gpsimd
                            dma_eng.dma_start(
                                rs_in[g][2048 * half:2048 * (half + 1)].rearrange(
                                    "(r p four) b -> p r (four b)", r=NCORE // 2, four=HPC),
                                pT_sb[:, 256 * half:256 * (half + 1)].rearrange(
                                    "p (r four b) -> p r (four b)", r=NCORE // 2, four=HPC),
                            )
                    nc.gpsimd.collective_compute(
                        "ReduceScatter", mybir.AluOpType.add,
                        replica_groups=[list(range(NCORE))],
                        ins=[rs_in[g].opt()], outs=[rs_out[g].opt()],
                    )
                    # y writeback on the SP ring: it waits on the collective,
                    # and SP has nothing queued behind it (ACT would stall its
                    # exp stream on this wait)
                    nc.sync.dma_start(y[:, 16 * g:16 * (g + 1)], rs_out[g][:])

    nc.finalize()
    return nc


_NC_CACHE = None


def _get_nc():
    global _NC_CACHE
    if _NC_CACHE is None:
        _NC_CACHE = _build_nc()
    return _NC_CACHE


def _prep_inputs(inputs):
    """Shard + lay out the full inputs for the 8 cores."""
    x = np.asarray(inputs["x"], np.float32)
    wq = np.asarray(inputs["wq"], np.float32)
    wk = np.asarray(inputs["wk"], np.float32)
    wv = np.asarray(inputs["wv"], np.float32)
    wo = np.asarray(inputs["wo"], np.float32)
    fc = np.asarray(inputs["freqs_cos"], np.float32)
    fs = np.asarray(inputs["freqs_sin"], np.float32)
    caches = (
        (np.asarray(inputs["cache_k0"], np.float32), np.asarray(inputs["cache_v0"], np.float32)),
        (np.asarray(inputs["cache_k1"], np.float32), np.asarray(inputs["cache_v1"], np.float32)),
    )

    x_flat = x.reshape(TOT_B, DIM)
    xh = np.ascontiguousarray(
        x_flat.T.reshape(KCH, 128, TOT_B).transpose(1, 0, 2)
    ).astype(NPDT)

    # RoPE tables: per-column position (2048 for tokens 0-15, 1024 for 16-31)
    C = np.empty((128, TOT_B), np.float32)
    S = np.empty((128, TOT_B), np.float32)
    for g in range(2):
        cos = fc[SP[g]]
        sin = fs[SP[g]]
        cols = slice(16 * g, 16 * (g + 1))
        C[0::2, cols] = cos[:, None]
        C[1::2, cols] = cos[:, None]
        S[0::2, cols] = -sin[:, None]
        S[1::2, cols] = sin[:, None]

    scale = 1.0 / math.sqrt(HEAD_DIM)
    in_maps = []
    for r in range(NCORE):
        w_q = wq[QF * r:QF * (r + 1)] * scale
        w_k = wk[HEAD_DIM * r:HEAD_DIM * (r + 1)]
        w_v = wv[HEAD_DIM * r:HEAD_DIM * (r + 1)]
        wqkvT = np.concatenate([w_q, w_k, w_v], axis=0).T  # [4096, 768]
        wqkv_hp = np.ascontiguousarray(
            wqkvT.reshape(KCH, 128, 768).transpose(1, 0, 2)
        ).astype(NPDT)

        # wo_cf[local_c, f] = wo[f, 512r + local_c]  -> [128, HPC, 4096]
        wo_cf = wo[:, QF * r:QF * (r + 1)].T  # [512, 4096]
        wo_hp = np.ascontiguousarray(
            wo_cf.reshape(HPC, 128, DIM).transpose(1, 0, 2)
        ).astype(NPDT)

        m = {"xh": xh, "wqkv": wqkv_hp, "wo": wo_hp,
             "ropec": C, "ropes": S}
        for g in range(2):
            ck, cv = caches[g]
            npos = SP[g]
            nf = NFULL[g]
            # cast to the wire dtype first, then do the layout copy at half width
            kslab = ck[:, :npos, r, :].astype(NPDT)       # [16, npos, 128]
            kt = np.ascontiguousarray(kslab.transpose(0, 2, 1))  # [16, 128, npos]
            vslab = cv[:, :npos, r, :].astype(NPDT).reshape(BSZ[g], nf, 128, HEAD_DIM)
            vp = np.empty((BSZ[g], 128, nf, 129), NPDT)
            vp[:, :, :, HEAD_DIM] = NPDT(1.0)
            vp[:, :, :, :HEAD_DIM] = vslab.transpose(0, 2, 1, 3)
            m[f"kt{g}"] = kt
            m[f"vp{g}"] = vp
        in_maps.append(m)
    return in_maps


def _run(inputs, trace=False):
    nc = _get_nc()
    in_maps = _prep_inputs(inputs)
    res = run_bass_kernel_spmd(nc, in_maps, core_ids=list(range(NCORE)), trace=trace)
    # each core returns yT rows [512r : 512r+512] of the [4096, 32] output,
    # row-permuted within the block (row = 4p + fb%4 -> f_local = 128*(fb%4) + p)
    parts = []
    for r in range(NCORE):
        yr = res.results[r]["y"]  # [512, 32]
        parts.append(yr.reshape(128, HPC, TOT_B).transpose(1, 0, 2).reshape(QF, TOT_B))
    y_t = np.concatenate(parts, axis=0)
    out = np.ascontiguousarray(y_t.T).reshape(TOT_B, 1, DIM).astype(np.float32)
    return out, res


def kernel(**inputs):
    out, _ = _run(inputs, trace=False)
    return out


# revision 48
# speedup vs baseline: 1.9366x; 1.0015x over previous
"""Trainium2 Bass kernel for GQA decode attention (nn_Attention_45844480917562).

Tensor-parallel over 8 NeuronCores: each core owns 4 query heads + 1 KV head
(wq/wk/wv column-sharded). The output projection is reduction-parallel: each
core computes its partial wo product transposed and a per-sample-group
ReduceScatter(add) leaves each core its own 512 output-feature rows; the host
only concatenates/transposes.

Compute dtype is bf16 (fp32 PSUM accumulation, fp32 softmax denominator /
division); BASS_ATTN_F32=1 switches to full fp32 at ~2x the HBM traffic.

Self-contained: hardcodes all shapes; host-side prep reshapes/transposes the
full inputs into per-core DMA-friendly layouts (K cache transposed to
[head_dim, pos], V cache chunk-major with a fused ones-column that yields the
softmax denominator for free in the P@V matmul).
"""

import os
import sys
import math

sys.path.insert(0, "/opt/trn_rl_repo")

import numpy as np
import ml_dtypes

import concourse.bass as bass
import concourse.mybir as mybir
from concourse import tile, bacc, masks
from concourse.bass_utils import run_bass_kernel_spmd

# ---------------- problem constants ----------------
DIM = 4096
N_HEADS = 32
N_KV_HEADS = 8
HEAD_DIM = 128
NCORE = 8
HPC = N_HEADS // NCORE            # 4 query heads per core
QF = HPC * HEAD_DIM               # 512 features per core
BSZ = (16, 16)
SP = (2048, 1024)                 # start_pos per group
TOT_B = 32
NFULL = (SP[0] // 128, SP[1] // 128)   # full 128-pos chunks per group: 16, 8
KCH = DIM // 128                  # 32 contraction chunks

USE_F32 = bool(int(os.environ.get("BASS_ATTN_F32", "0")))
DT = mybir.dt.float32 if USE_F32 else mybir.dt.bfloat16
NPDT = np.float32 if USE_F32 else ml_dtypes.bfloat16
SPT = 1 if USE_F32 else 4          # samples per KV tile (f32 tiles are 2x bytes)
WQ_BUFS = 2 if USE_F32 else 4

f32 = mybir.dt.float32


def _build_nc():
    nc = bacc.Bacc(trn_type="TRN2", num_devices=NCORE, enable_asserts=True)

    # ---- I/O ----
    xh = nc.dram_tensor("xh", [128, KCH, TOT_B], DT, kind="ExternalInput")
    wqkv = nc.dram_tensor("wqkv", [128, KCH, QF + 2 * HEAD_DIM], DT, kind="ExternalInput")
    # wo in [local_c, f] layout: wo_cf[p, h, f] = wo[f, 512*r + h*128 + p]
    wo = nc.dram_tensor("wo", [128, HPC, DIM], DT, kind="ExternalInput")
    kt0 = nc.dram_tensor("kt0", [BSZ[0], 128, SP[0]], DT, kind="ExternalInput")
    kt1 = nc.dram_tensor("kt1", [BSZ[1], 128, SP[1]], DT, kind="ExternalInput")
    vp0 = nc.dram_tensor("vp0", [BSZ[0], 128, NFULL[0], 129], DT, kind="ExternalInput")
    vp1 = nc.dram_tensor("vp1", [BSZ[1], 128, NFULL[1], 129], DT, kind="ExternalInput")
    ropec = nc.dram_tensor("ropec", [128, TOT_B], f32, kind="ExternalInput")
    ropes = nc.dram_tensor("ropes", [128, TOT_B], f32, kind="ExternalInput")
    # yT: rows = this core's 512 output features (f = 512*r + row), cols = samples
    y = nc.dram_tensor("y", [QF, TOT_B], f32, kind="ExternalOutput")

    WQKV_W = QF + 2 * HEAD_DIM  # 768
    SWAP_MASK = [i ^ 1 for i in range(32)]

    with tile.TileContext(nc) as tc:
        with tc.tile_pool(name="cpool", bufs=1) as cpool, \
             tc.tile_pool(name="wpool", bufs=2) as wpool, \
             tc.tile_pool(name="kvpool", bufs=2) as kvpool, \
             tc.tile_pool(name="apool", bufs=3) as apool, \
             tc.tile_pool(name="ps_t", bufs=2, space="PSUM") as ps_t, \
             tc.tile_pool(name="dpool", bufs=1, space="DRAM") as dpool:

            # ---------- constants ----------
            ident = cpool.tile([128, 128], f32)
            masks.make_identity(nc, ident[:])
            identdt = cpool.tile([TOT_B, TOT_B], DT)
            masks.make_identity(nc, identdt[:])

            # x + wqkv go at the head of the SP ring (same ring as the KV
            # stream) so the QKV critical chain gets full DMA bandwidth
            # before the bulk KV traffic.
            x_sb = cpool.tile([128, KCH * TOT_B], DT)
            nc.sync.dma_start(x_sb[:].rearrange("p (c b) -> p c b", c=KCH), xh[:])
            ropec_sb = cpool.tile([128, TOT_B], f32)
            nc.scalar.dma_start(ropec_sb[:], ropec[:])
            ropes_sb = cpool.tile([128, TOT_B], f32)
            nc.scalar.dma_start(ropes_sb[:], ropes[:])

            # ---------- phase A: QKV projection ----------
            with tc.tile_pool(name="ps_a", bufs=1, space="PSUM") as ps_a:
                qkv_ps = ps_a.tile([TOT_B, WQKV_W], f32)
                for P in range(4):
                    wq_t = wpool.tile([128, 8 * WQKV_W], DT, tag="wq", bufs=WQ_BUFS)
                    nc.sync.dma_start(
                        wq_t[:].rearrange("p (c j) -> p c j", c=8),
                        wqkv[:, 8 * P:8 * P + 8, :],
                    )
                    for ci in range(8):
                        c = 8 * P + ci
                        lhs = x_sb[:, TOT_B * c:TOT_B * (c + 1)]
                        rhs = wq_t[:, WQKV_W * ci:WQKV_W * (ci + 1)]
                        nc.tensor.matmul(qkv_ps[:, 0:512], lhs, rhs[:, 0:512],
                                         start=(c == 0), stop=(c == KCH - 1))
                        nc.tensor.matmul(qkv_ps[:, 512:768], lhs, rhs[:, 512:768],
                                         start=(c == 0), stop=(c == KCH - 1))

                qkv_sb = cpool.tile([TOT_B, WQKV_W], f32)
                nc.scalar.copy(qkv_sb[:], qkv_ps[:])

            # wo weights prefetch tile; the DMA is issued mid-stream (after
            # group 0's KV loads are queued) on the ACT ring
            wo_all = wpool.tile([128, KCH * QF], DT, tag="wo", bufs=1)

            # new-position V (plus ones column for the softmax denominator)
            vnew = cpool.tile([TOT_B, 129], DT)
            nc.vector.tensor_copy(vnew[:, 0:HEAD_DIM], qkv_sb[:, 640:768])
            nc.vector.memset(vnew[:, 128:129], 1.0)

            # ---------- transpose q heads + k, apply RoPE ----------
            qT4 = cpool.tile([128, HPC * TOT_B], DT)   # col = b*4 + h
            kTn = cpool.tile([128, TOT_B], DT)         # col = b
            for h in range(HPC + 1):                   # 4 q heads then k
                tp = ps_t.tile([128, TOT_B], f32, tag="tp")
                nc.tensor.transpose(tp[:], qkv_sb[:, 128 * h:128 * (h + 1)],
                                    ident[0:TOT_B, 0:TOT_B])
                t_sb = apool.tile([128, TOT_B], f32, tag="tr")
                nc.vector.tensor_copy(t_sb[:], tp[:])
                sw = apool.tile([128, TOT_B], f32, tag="sw")
                nc.vector.stream_shuffle(sw[:], t_sb[:], SWAP_MASK)
                t1 = apool.tile([128, TOT_B], f32, tag="t1")
                nc.vector.tensor_mul(t1[:], t_sb[:], ropec_sb[:])
                nc.vector.tensor_mul(sw[:], sw[:], ropes_sb[:])
                if h < HPC:
                    dest = qT4[:, h::HPC]
                else:
                    dest = kTn[:]
                nc.vector.tensor_add(dest, t1[:], sw[:])

            # ---------- phase B: attention over the KV cache ----------
            attnT = cpool.tile([128, HPC * TOT_B], DT)  # col = h*32 + b
            kts = (kt0, kt1)
            vps = (vp0, vp1)
            # Output projection is reduction-parallel: each core computes its
            # partial wo product (transposed, [4096, 16] per sample group) and
            # a ReduceScatter(add) sums across cores, leaving each core its
            # own 512 output-feature rows. Split per sample-group so the
            # first collective overlaps the second group's attention.
            rs_in = [dpool.tile([DIM, 16], f32, name=f"rs_in{g}") for g in range(2)]
            rs_out = [dpool.tile([QF, 16], f32, name=f"rs_out{g}")
                      for g in range(2)]
            with tc.tile_pool(name="ps_b", bufs=2, space="PSUM") as ps_b:
                for g in range(2):
                    npos = SP[g]
                    nf = NFULL[g]
                    ncol = 4 * nf
                    vw = 129 * nf
                    # taper the tail tiles of the LAST group so the serial
                    # per-sample attention chain after the final DMA byte is
                    # as short as possible
                    if SPT == 1:
                        blocks = [1] * BSZ[g]
                    elif g == 1:
                        blocks = [4, 4, 4, 2, 1, 1]
                    else:
                        blocks = [SPT] * (BSZ[g] // SPT)
                    s_off = 0
                    for blk in blocks:
                        ktile = kvpool.tile([128, SPT * SP[0]], DT, tag="kt")
                        nc.sync.dma_start(
                            ktile[:, 0:blk * npos].rearrange("p (s n) -> p s n", s=blk),
                            kts[g][s_off:s_off + blk].rearrange("s p n -> p s n"),
                        )
                        vtile = kvpool.tile([128, SPT * 129 * NFULL[0]], DT, tag="vt")
                        nc.sync.dma_start(
                            vtile[:, 0:blk * vw].rearrange("p (s c d) -> p s c d", s=blk, c=nf),
                            vps[g][s_off:s_off + blk].rearrange("s p c d -> p s c d"),
                        )
                        for j in range(blk):
                            b = 16 * g + s_off + j
                            ks = ktile[:, j * npos:(j + 1) * npos]
                            vs = vtile[:, j * vw:(j + 1) * vw]
                            q_b = qT4[:, HPC * b:HPC * (b + 1)]

                            sc_ps = ps_b.tile([128, 68], f32, tag="sc")
                            for c in range(nf):
                                nc.tensor.matmul(sc_ps[:, 4 * c:4 * c + 4],
                                                 ks[:, 128 * c:128 * (c + 1)], q_b,
                                                 start=True, stop=True)
                            nc.tensor.matmul(sc_ps[0:1, ncol:ncol + 4],
                                             kTn[:, b:b + 1], q_b,
                                             start=True, stop=True)

                            pr = apool.tile([128, 68], DT, tag="pr")
                            nc.scalar.activation(pr[:, 0:ncol], sc_ps[:, 0:ncol],
                                                 mybir.ActivationFunctionType.Exp)
                            nc.scalar.activation(pr[0:1, ncol:ncol + 4],
                                                 sc_ps[0:1, ncol:ncol + 4],
                                                 mybir.ActivationFunctionType.Exp)

                            # select row b of vnew into partition 0 (psum), for the
                            # tail matmul rhs (moving operand must be partition-0 based)
                            vrow_ps = ps_b.tile([1, 129], f32, tag="vr", bufs=1)
                            nc.tensor.matmul(vrow_ps[:], identdt[:, b:b + 1], vnew[:],
                                             start=True, stop=True)
                            vrow = apool.tile([1, 129], DT, tag="vrow")
                            nc.vector.tensor_copy(vrow[:], vrow_ps[:])

                            o_ps = ps_b.tile([HPC, 129], f32, tag="o")
                            for c in range(nf):
                                nc.tensor.matmul(o_ps[:], pr[:, 4 * c:4 * c + 4],
                                                 vs[:, 129 * c:129 * (c + 1)],
                                                 start=(c == 0), stop=False)
                            nc.tensor.matmul(o_ps[:], pr[0:1, ncol:ncol + 4],
                                             vrow[:], start=False, stop=True)

                            rec = apool.tile([HPC, 1], f32, tag="rec")
                            nc.vector.reciprocal(rec[:], o_ps[:, 128:129])
                            at = apool.tile([HPC, HEAD_DIM], f32, tag="at")
                            nc.vector.tensor_scalar_mul(at[:], o_ps[:, 0:HEAD_DIM], rec[:])

                            tp2 = ps_t.tile([128, TOT_B], f32, tag="tp")
                            nc.tensor.transpose(tp2[:, 0:HPC], at[:], ident[0:HPC, 0:HPC])
                            nc.vector.tensor_copy(attnT[:, b::TOT_B], tp2[:, 0:HPC])
                        s_off += blk

                    if g == 0:
                        nc.scalar.dma_start(
                            wo_all[:].rearrange("p (c j) -> p c j", c=HPC), wo[:])

                    # this group's samples are done: partial wo product
                    # partialT[f, b] = sum_c wo[f, c] * attn[b, c]  (c = own features)
                    pT_sb = apool.tile([128, 32 * 16], f32, tag="pt", bufs=2)
                    for fq in range(8):          # 4 fb blocks per PSUM bank
                        pt_ps = ps_t.tile([128, 64], f32, tag="tp")
                        for fi in range(4):
                            fb = 4 * fq + fi
                            for h in range(HPC):
                                nc.tensor.matmul(
                                    pt_ps[:, 16 * fi:16 * (fi + 1)],
                                    wo_all[:, h * DIM + 128 * fb:h * DIM + 128 * (fb + 1)],
                                    attnT[:, TOT_B * h + 16 * g:TOT_B * h + 16 * (g + 1)],
                                    start=(h == 0), stop=(h == HPC - 1))
                        nc.vector.tensor_copy(pT_sb[:, 64 * fq:64 * (fq + 1)], pt_ps[:])
                        # rs_in row order is permuted within each rank's
                        # 512-row block (row = 512r + 4p + fb%4) so DRAM
                        # writes are 256B contiguous runs instead of 64B;
                        # host un-permutes. Shipped in two rank-halves so the
                        # first overlaps the remaining partial matmuls.
                        if fq == 3 or fq == 7:
                            half = fq // 4
                            # group 1's rs_in is tail-critical: use the idle
                            # ACT HWDGE ring (faster first-byte than SWDGE);
                            # group 0's stays on gpsimd mid-stream.
                            dma_eng = nc.scalar if g == 1 else nc.gpsimd
                            dma_eng.dma_start(
                                rs_in[g][2048 * half:2048 * (half + 1)].rearrange(
                                    "(r p four) b -> p r (four b)", r=NCORE // 2, four=HPC),
                                pT_sb[:, 256 * half:256 * (half + 1)].rearrange(
                                    "p (r four b) -> p r (four b)", r=NCORE // 2, four=HPC),
                            )
                    nc.gpsimd.collective_compute(
                        "ReduceScatter", mybir.AluOpType.add,
                        replica_groups=[list(range(NCORE))],
                        ins=[rs_in[g].opt()], outs=[rs_out[g].opt()],
                    )
                    # y writeback on the SP ring: it waits on the collective,
                    # and SP has nothing queued behind it (ACT would stall its
                    # exp stream on this wait)
                    nc.sync.dma_start(y[:, 16 * g:16 * (g + 1)], rs_out[g][:])

    nc.finalize()
    return nc


_NC_CACHE = None


def _get_nc():
    global _NC_CACHE
    if _NC_CACHE is None:
        _NC_CACHE = _build_nc()
    return _NC_CACHE


def _prep_inputs(inputs):
    """Shard + lay out the full inputs for the 8 cores."""
    x = np.asarray(inputs["x"], np.float32)
    wq = np.asarray(inputs["wq"], np.float32)
    wk = np.asarray(inputs["wk"], np.float32)
    wv = np.asarray(inputs["wv"], np.float32)
    wo = np.asarray(inputs["wo"], np.float32)
    fc = np.asarray(inputs["freqs_cos"], np.float32)
    fs = np.asarray(inputs["freqs_sin"], np.float32)
    caches = (
        (np.asarray(inputs["cache_k0"], np.float32), np.asarray(inputs["cache_v0"], np.float32)),
        (np.asarray(inputs["cache_k1"], np.float32), np.asarray(inputs["cache_v1"], np.float32)),
    )

    x_flat = x.reshape(TOT_B, DIM)
    xh = np.ascontiguousarray(
        x_flat.T.reshape(KCH, 128, TOT_B).transpose(1, 0, 2)
    ).astype(NPDT)

    # RoPE tables: per-column position (2048 for tokens 0-15, 1024 for 16-31)
    C = np.empty((128, TOT_B), np.float32)
    S = np.empty((128, TOT_B), np.float32)
    for g in range(2):
        cos = fc[SP[g]]
        sin = fs[SP[g]]
        cols = slice(16 * g, 16 * (g + 1))
        C[0::2, cols] = cos[:, None]
        C[1::2, cols] = cos[:, None]
        S[0::2, cols] = -sin[:, None]
        S[1::2, cols] = sin[:, None]

    scale = 1.0 / math.sqrt(HEAD_DIM)
    in_maps = []
    for r in range(NCORE):
        w_q = wq[QF * r:QF * (r + 1)] * scale
        w_k = wk[HEAD_DIM * r:HEAD_DIM * (r + 1)]
        w_v = wv[HEAD_DIM * r:HEAD_DIM * (r + 1)]
        wqkvT = np.concatenate([w_q, w_k, w_v], axis=0).T  # [4096, 768]
        wqkv_hp = np.ascontiguousarray(
            wqkvT.reshape(KCH, 128, 768).transpose(1, 0, 2)
        ).astype(NPDT)

        # wo_cf[local_c, f] = wo[f, 512r + local_c]  -> [128, HPC, 4096]
        wo_cf = wo[:, QF * r:QF * (r + 1)].T  # [512, 4096]
        wo_hp = np.ascontiguousarray(
            wo_cf.reshape(HPC, 128, DIM).transpose(1, 0, 2)
        ).astype(NPDT)

        m = {"xh": xh, "wqkv": wqkv_hp, "wo": wo_hp,
             "ropec": C, "ropes": S}
        for g in range(2):
            ck, cv = caches[g]
            npos = SP[g]
            nf = NFULL[g]
            # cast to the wire dtype first, then do the layout copy at half width
            kslab = ck[:, :npos, r, :].astype(NPDT)       # [16, npos, 128]
            kt = np.ascontiguousarray(kslab.transpose(0, 2, 1))  # [16, 128, npos]
            vslab = cv[:, :npos, r, :].astype(NPDT).reshape(BSZ[g], nf, 128, HEAD_DIM)
            vp = np.empty((BSZ[g], 128, nf, 129), NPDT)
            vp[:, :, :, HEAD_DIM] = NPDT(1.0)
            vp[:, :, :, :HEAD_DIM] = vslab.transpose(0, 2, 1, 3)
            m[f"kt{g}"] = kt
            m[f"vp{g}"] = vp
        in_maps.append(m)
    return in_maps


def _run(inputs, trace=False):
    nc = _get_nc()
    in_maps = _prep_inputs(inputs)
    res = run_bass_kernel_spmd(nc, in_maps, core_ids=list(range(NCORE)), trace=trace)
    # each core returns yT rows [512r : 512r+512] of the [4096, 32] output,
    # row-permuted within the block (row = 4p + fb%4 -> f_local = 128*(fb%4) + p)
    parts = []
    for r in range(NCORE):
        yr = res.results[r]["y"]  # [512, 32]
        parts.append(yr.reshape(128, HPC, TOT_B).transpose(1, 0, 2).reshape(QF, TOT_B))
    y_t = np.concatenate(parts, axis=0)
    out = np.ascontiguousarray(y_t.T).reshape(TOT_B, 1, DIM).astype(np.float32)
    return out, res


def kernel(**inputs):
    out, _ = _run(inputs, trace=False)
    return out


# revision 49
# speedup vs baseline: 1.9574x; 1.0107x over previous
"""Trainium2 Bass kernel for GQA decode attention (nn_Attention_45844480917562).

Tensor-parallel over 8 NeuronCores: each core owns 4 query heads + 1 KV head
(wq/wk/wv column-sharded). The output projection is reduction-parallel: each
core computes its partial wo product transposed and a per-sample-group
ReduceScatter(add) leaves each core its own 512 output-feature rows; the host
only concatenates/transposes.

Compute dtype is bf16 (fp32 PSUM accumulation, fp32 softmax denominator /
division); BASS_ATTN_F32=1 switches to full fp32 at ~2x the HBM traffic.

Self-contained: hardcodes all shapes; host-side prep reshapes/transposes the
full inputs into per-core DMA-friendly layouts (K cache transposed to
[head_dim, pos], V cache chunk-major with a fused ones-column that yields the
softmax denominator for free in the P@V matmul).
"""

import os
import sys
import math

sys.path.insert(0, "/opt/trn_rl_repo")

import numpy as np
import ml_dtypes

import concourse.bass as bass
import concourse.mybir as mybir
from concourse import tile, bacc, masks
from concourse.bass_utils import run_bass_kernel_spmd

# ---------------- problem constants ----------------
DIM = 4096
N_HEADS = 32
N_KV_HEADS = 8
HEAD_DIM = 128
NCORE = 8
HPC = N_HEADS // NCORE            # 4 query heads per core
QF = HPC * HEAD_DIM               # 512 features per core
BSZ = (16, 16)
SP = (2048, 1024)                 # start_pos per group
TOT_B = 32
NFULL = (SP[0] // 128, SP[1] // 128)   # full 128-pos chunks per group: 16, 8
KCH = DIM // 128                  # 32 contraction chunks

USE_F32 = bool(int(os.environ.get("BASS_ATTN_F32", "0")))
DT = mybir.dt.float32 if USE_F32 else mybir.dt.bfloat16
NPDT = np.float32 if USE_F32 else ml_dtypes.bfloat16
SPT = 1 if USE_F32 else 4          # samples per KV tile (f32 tiles are 2x bytes)
WQ_BUFS = 2 if USE_F32 else 4

f32 = mybir.dt.float32


def _build_nc():
    nc = bacc.Bacc(trn_type="TRN2", num_devices=NCORE, enable_asserts=True)

    # ---- I/O ----
    xh = nc.dram_tensor("xh", [128, KCH, TOT_B], DT, kind="ExternalInput")
    wqkv = nc.dram_tensor("wqkv", [128, KCH, QF + 2 * HEAD_DIM], DT, kind="ExternalInput")
    # wo in [local_c, f] layout: wo_cf[p, h, f] = wo[f, 512*r + h*128 + p]
    wo = nc.dram_tensor("wo", [128, HPC, DIM], DT, kind="ExternalInput")
    kt0 = nc.dram_tensor("kt0", [BSZ[0], 128, SP[0]], DT, kind="ExternalInput")
    kt1 = nc.dram_tensor("kt1", [BSZ[1], 128, SP[1]], DT, kind="ExternalInput")
    vp0 = nc.dram_tensor("vp0", [BSZ[0], 128, NFULL[0], 129], DT, kind="ExternalInput")
    vp1 = nc.dram_tensor("vp1", [BSZ[1], 128, NFULL[1], 129], DT, kind="ExternalInput")
    ropec = nc.dram_tensor("ropec", [128, TOT_B], f32, kind="ExternalInput")
    ropes = nc.dram_tensor("ropes", [128, TOT_B], f32, kind="ExternalInput")
    # yT: rows = this core's 512 output features (f = 512*r + row), cols = samples
    y = nc.dram_tensor("y", [QF, TOT_B], f32, kind="ExternalOutput")

    WQKV_W = QF + 2 * HEAD_DIM  # 768
    SWAP_MASK = [i ^ 1 for i in range(32)]

    with tile.TileContext(nc) as tc:
        with tc.tile_pool(name="cpool", bufs=1) as cpool, \
             tc.tile_pool(name="wpool", bufs=2) as wpool, \
             tc.tile_pool(name="kvpool", bufs=2) as kvpool, \
             tc.tile_pool(name="apool", bufs=3) as apool, \
             tc.tile_pool(name="ps_t", bufs=3, space="PSUM") as ps_t, \
             tc.tile_pool(name="dpool", bufs=1, space="DRAM") as dpool:

            # ---------- constants ----------
            ident = cpool.tile([128, 128], f32)
            masks.make_identity(nc, ident[:])
            identdt = cpool.tile([TOT_B, TOT_B], DT)
            masks.make_identity(nc, identdt[:])

            # x + wqkv go at the head of the SP ring (same ring as the KV
            # stream) so the QKV critical chain gets full DMA bandwidth
            # before the bulk KV traffic.
            x_sb = cpool.tile([128, KCH * TOT_B], DT)
            nc.sync.dma_start(x_sb[:].rearrange("p (c b) -> p c b", c=KCH), xh[:])
            ropec_sb = cpool.tile([128, TOT_B], f32)
            nc.scalar.dma_start(ropec_sb[:], ropec[:])
            ropes_sb = cpool.tile([128, TOT_B], f32)
            nc.scalar.dma_start(ropes_sb[:], ropes[:])

            # ---------- phase A: QKV projection ----------
            with tc.tile_pool(name="ps_a", bufs=1, space="PSUM") as ps_a:
                qkv_ps = ps_a.tile([TOT_B, WQKV_W], f32)
                for P in range(4):
                    wq_t = wpool.tile([128, 8 * WQKV_W], DT, tag="wq", bufs=WQ_BUFS)
                    nc.sync.dma_start(
                        wq_t[:].rearrange("p (c j) -> p c j", c=8),
                        wqkv[:, 8 * P:8 * P + 8, :],
                    )
                    for ci in range(8):
                        c = 8 * P + ci
                        lhs = x_sb[:, TOT_B * c:TOT_B * (c + 1)]
                        rhs = wq_t[:, WQKV_W * ci:WQKV_W * (ci + 1)]
                        nc.tensor.matmul(qkv_ps[:, 0:512], lhs, rhs[:, 0:512],
                                         start=(c == 0), stop=(c == KCH - 1))
                        nc.tensor.matmul(qkv_ps[:, 512:768], lhs, rhs[:, 512:768],
                                         start=(c == 0), stop=(c == KCH - 1))

                qkv_sb = cpool.tile([TOT_B, WQKV_W], f32)
                nc.scalar.copy(qkv_sb[:], qkv_ps[:])

            # wo weights prefetch tile; the DMA is issued mid-stream (after
            # group 0's KV loads are queued) on the ACT ring
            wo_all = wpool.tile([128, KCH * QF], DT, tag="wo", bufs=1)

            # new-position V (plus ones column for the softmax denominator)
            vnew = cpool.tile([TOT_B, 129], DT)
            nc.vector.tensor_copy(vnew[:, 0:HEAD_DIM], qkv_sb[:, 640:768])
            nc.vector.memset(vnew[:, 128:129], 1.0)

            # ---------- transpose q heads + k, apply RoPE ----------
            qT4 = cpool.tile([128, HPC * TOT_B], DT)   # col = b*4 + h
            kTn = cpool.tile([128, TOT_B], DT)         # col = b
            for h in range(HPC + 1):                   # 4 q heads then k
                tp = ps_t.tile([128, TOT_B], f32, tag="tp")
                nc.tensor.transpose(tp[:], qkv_sb[:, 128 * h:128 * (h + 1)],
                                    ident[0:TOT_B, 0:TOT_B])
                t_sb = apool.tile([128, TOT_B], f32, tag="tr")
                nc.vector.tensor_copy(t_sb[:], tp[:])
                sw = apool.tile([128, TOT_B], f32, tag="sw")
                nc.vector.stream_shuffle(sw[:], t_sb[:], SWAP_MASK)
                t1 = apool.tile([128, TOT_B], f32, tag="t1")
                nc.vector.tensor_mul(t1[:], t_sb[:], ropec_sb[:])
                nc.vector.tensor_mul(sw[:], sw[:], ropes_sb[:])
                if h < HPC:
                    dest = qT4[:, h::HPC]
                else:
                    dest = kTn[:]
                nc.vector.tensor_add(dest, t1[:], sw[:])

            # ---------- phase B: attention over the KV cache ----------
            attnT = cpool.tile([128, HPC * TOT_B], DT)  # col = h*32 + b
            kts = (kt0, kt1)
            vps = (vp0, vp1)
            # Output projection is reduction-parallel: each core computes its
            # partial wo product (transposed, [4096, 16] per sample group) and
            # a ReduceScatter(add) sums across cores, leaving each core its
            # own 512 output-feature rows. Split per sample-group so the
            # first collective overlaps the second group's attention.
            rs_in = [dpool.tile([DIM, 16], f32, name=f"rs_in{g}") for g in range(2)]
            rs_out = [dpool.tile([QF, 16], f32, name=f"rs_out{g}")
                      for g in range(2)]
            with tc.tile_pool(name="ps_b", bufs=2, space="PSUM") as ps_b:
                for g in range(2):
                    npos = SP[g]
                    nf = NFULL[g]
                    ncol = 4 * nf
                    vw = 129 * nf
                    # taper the tail tiles of the LAST group so the serial
                    # per-sample attention chain after the final DMA byte is
                    # as short as possible
                    if SPT == 1:
                        blocks = [1] * BSZ[g]
                    elif g == 1:
                        blocks = [4, 4, 4, 2, 1, 1]
                    else:
                        blocks = [SPT] * (BSZ[g] // SPT)
                    s_off = 0
                    for blk in blocks:
                        ktile = kvpool.tile([128, SPT * SP[0]], DT, tag="kt")
                        nc.sync.dma_start(
                            ktile[:, 0:blk * npos].rearrange("p (s n) -> p s n", s=blk),
                            kts[g][s_off:s_off + blk].rearrange("s p n -> p s n"),
                        )
                        vtile = kvpool.tile([128, SPT * 129 * NFULL[0]], DT, tag="vt")
                        nc.sync.dma_start(
                            vtile[:, 0:blk * vw].rearrange("p (s c d) -> p s c d", s=blk, c=nf),
                            vps[g][s_off:s_off + blk].rearrange("s p c d -> p s c d"),
                        )
                        for j in range(blk):
                            b = 16 * g + s_off + j
                            ks = ktile[:, j * npos:(j + 1) * npos]
                            vs = vtile[:, j * vw:(j + 1) * vw]
                            q_b = qT4[:, HPC * b:HPC * (b + 1)]

                            sc_ps = ps_b.tile([128, 68], f32, tag="sc")
                            for c in range(nf):
                                nc.tensor.matmul(sc_ps[:, 4 * c:4 * c + 4],
                                                 ks[:, 128 * c:128 * (c + 1)], q_b,
                                                 start=True, stop=True)
                            nc.tensor.matmul(sc_ps[0:1, ncol:ncol + 4],
                                             kTn[:, b:b + 1], q_b,
                                             start=True, stop=True)

                            pr = apool.tile([128, 68], DT, tag="pr")
                            nc.scalar.activation(pr[:, 0:ncol], sc_ps[:, 0:ncol],
                                                 mybir.ActivationFunctionType.Exp)
                            nc.scalar.activation(pr[0:1, ncol:ncol + 4],
                                                 sc_ps[0:1, ncol:ncol + 4],
                                                 mybir.ActivationFunctionType.Exp)

                            # select row b of vnew into partition 0 (psum), for the
                            # tail matmul rhs (moving operand must be partition-0 based)
                            vrow_ps = ps_b.tile([1, 129], f32, tag="vr", bufs=1)
                            nc.tensor.matmul(vrow_ps[:], identdt[:, b:b + 1], vnew[:],
                                             start=True, stop=True)
                            vrow = apool.tile([1, 129], DT, tag="vrow")
                            nc.vector.tensor_copy(vrow[:], vrow_ps[:])

                            o_ps = ps_b.tile([HPC, 129], f32, tag="o")
                            for c in range(nf):
                                nc.tensor.matmul(o_ps[:], pr[:, 4 * c:4 * c + 4],
                                                 vs[:, 129 * c:129 * (c + 1)],
                                                 start=(c == 0), stop=False)
                            nc.tensor.matmul(o_ps[:], pr[0:1, ncol:ncol + 4],
                                             vrow[:], start=False, stop=True)

                            rec = apool.tile([HPC, 1], f32, tag="rec")
                            nc.vector.reciprocal(rec[:], o_ps[:, 128:129])
                            at = apool.tile([HPC, HEAD_DIM], f32, tag="at")
                            nc.vector.tensor_scalar_mul(at[:], o_ps[:, 0:HEAD_DIM], rec[:])

                            tp2 = ps_t.tile([128, TOT_B], f32, tag="tp")
                            nc.tensor.transpose(tp2[:, 0:HPC], at[:], ident[0:HPC, 0:HPC])
                            nc.vector.tensor_copy(attnT[:, b::TOT_B], tp2[:, 0:HPC])
                        s_off += blk

                    if g == 0:
                        nc.scalar.dma_start(
                            wo_all[:].rearrange("p (c j) -> p c j", c=HPC), wo[:])

                    # this group's samples are done: partial wo product
                    # partialT[f, b] = sum_c wo[f, c] * attn[b, c]  (c = own features)
                    pT_sb = apool.tile([128, 32 * 16], f32, tag="pt", bufs=2)
                    for fq in range(8):          # 4 fb blocks per PSUM bank
                        pt_ps = ps_t.tile([128, 64], f32, tag="tp")
                        for fi in range(4):
                            fb = 4 * fq + fi
                            for h in range(HPC):
                                nc.tensor.matmul(
                                    pt_ps[:, 16 * fi:16 * (fi + 1)],
                                    wo_all[:, h * DIM + 128 * fb:h * DIM + 128 * (fb + 1)],
                                    attnT[:, TOT_B * h + 16 * g:TOT_B * h + 16 * (g + 1)],
                                    start=(h == 0), stop=(h == HPC - 1))
                        nc.vector.tensor_copy(pT_sb[:, 64 * fq:64 * (fq + 1)], pt_ps[:])
                        # rs_in row order is permuted within each rank's
                        # 512-row block (row = 512r + 4p + fb%4) so DRAM
                        # writes are 256B contiguous runs instead of 64B;
                        # host un-permutes. Shipped in two rank-halves so the
                        # first overlaps the remaining partial matmuls.
                        if fq == 3 or fq == 7:
                            half = fq // 4
                            # group 1's rs_in is tail-critical: use the idle
                            # ACT HWDGE ring (faster first-byte than SWDGE);
                            # group 0's stays on gpsimd mid-stream.
                            dma_eng = nc.scalar if g == 1 else nc.gpsimd
                            dma_eng.dma_start(
                                rs_in[g][2048 * half:2048 * (half + 1)].rearrange(
                                    "(r p four) b -> p r (four b)", r=NCORE // 2, four=HPC),
                                pT_sb[:, 256 * half:256 * (half + 1)].rearrange(
                                    "p (r four b) -> p r (four b)", r=NCORE // 2, four=HPC),
                            )
                    nc.gpsimd.collective_compute(
                        "ReduceScatter", mybir.AluOpType.add,
                        replica_groups=[list(range(NCORE))],
                        ins=[rs_in[g].opt()], outs=[rs_out[g].opt()],
                    )
                    # y writeback on the SP ring: it waits on the collective,
                    # and SP has nothing queued behind it (ACT would stall its
                    # exp stream on this wait)
                    nc.sync.dma_start(y[:, 16 * g:16 * (g + 1)], rs_out[g][:])

    nc.finalize()
    return nc


_NC_CACHE = None


def _get_nc():
    global _NC_CACHE
    if _NC_CACHE is None:
        _NC_CACHE = _build_nc()
    return _NC_CACHE


def _prep_inputs(inputs):
    """Shard + lay out the full inputs for the 8 cores."""
    x = np.asarray(inputs["x"], np.float32)
    wq = np.asarray(inputs["wq"], np.float32)
    wk = np.asarray(inputs["wk"], np.float32)
    wv = np.asarray(inputs["wv"], np.float32)
    wo = np.asarray(inputs["wo"], np.float32)
    fc = np.asarray(inputs["freqs_cos"], np.float32)
    fs = np.asarray(inputs["freqs_sin"], np.float32)
    caches = (
        (np.asarray(inputs["cache_k0"], np.float32), np.asarray(inputs["cache_v0"], np.float32)),
        (np.asarray(inputs["cache_k1"], np.float32), np.asarray(inputs["cache_v1"], np.float32)),
    )

    x_flat = x.reshape(TOT_B, DIM)
    xh = np.ascontiguousarray(
        x_flat.T.reshape(KCH, 128, TOT_B).transpose(1, 0, 2)
    ).astype(NPDT)

    # RoPE tables: per-column position (2048 for tokens 0-15, 1024 for 16-31)
    C = np.empty((128, TOT_B), np.float32)
    S = np.empty((128, TOT_B), np.float32)
    for g in range(2):
        cos = fc[SP[g]]
        sin = fs[SP[g]]
        cols = slice(16 * g, 16 * (g + 1))
        C[0::2, cols] = cos[:, None]
        C[1::2, cols] = cos[:, None]
        S[0::2, cols] = -sin[:, None]
        S[1::2, cols] = sin[:, None]

    scale = 1.0 / math.sqrt(HEAD_DIM)
    in_maps = []
    for r in range(NCORE):
        w_q = wq[QF * r:QF * (r + 1)] * scale
        w_k = wk[HEAD_DIM * r:HEAD_DIM * (r + 1)]
        w_v = wv[HEAD_DIM * r:HEAD_DIM * (r + 1)]
        wqkvT = np.concatenate([w_q, w_k, w_v], axis=0).T  # [4096, 768]
        wqkv_hp = np.ascontiguousarray(
            wqkvT.reshape(KCH, 128, 768).transpose(1, 0, 2)
        ).astype(NPDT)

        # wo_cf[local_c, f] = wo[f, 512r + local_c]  -> [128, HPC, 4096]
        wo_cf = wo[:, QF * r:QF * (r + 1)].T  # [512, 4096]
        wo_hp = np.ascontiguousarray(
            wo_cf.reshape(HPC, 128, DIM).transpose(1, 0, 2)
        ).astype(NPDT)

        m = {"xh": xh, "wqkv": wqkv_hp, "wo": wo_hp,
             "ropec": C, "ropes": S}
        for g in range(2):
            ck, cv = caches[g]
            npos = SP[g]
            nf = NFULL[g]
            # cast to the wire dtype first, then do the layout copy at half width
            kslab = ck[:, :npos, r, :].astype(NPDT)       # [16, npos, 128]
            kt = np.ascontiguousarray(kslab.transpose(0, 2, 1))  # [16, 128, npos]
            vslab = cv[:, :npos, r, :].astype(NPDT).reshape(BSZ[g], nf, 128, HEAD_DIM)
            vp = np.empty((BSZ[g], 128, nf, 129), NPDT)
            vp[:, :, :, HEAD_DIM] = NPDT(1.0)
            vp[:, :, :, :HEAD_DIM] = vslab.transpose(0, 2, 1, 3)
            m[f"kt{g}"] = kt
            m[f"vp{g}"] = vp
        in_maps.append(m)
    return in_maps


def _run(inputs, trace=False):
    nc = _get_nc()
    in_maps = _prep_inputs(inputs)
    res = run_bass_kernel_spmd(nc, in_maps, core_ids=list(range(NCORE)), trace=trace)
    # each core returns yT rows [512r : 512r+512] of the [4096, 32] output,
    # row-permuted within the block (row = 4p + fb%4 -> f_local = 128*(fb%4) + p)
    parts = []
    for r in range(NCORE):
        yr = res.results[r]["y"]  # [512, 32]
        parts.append(yr.reshape(128, HPC, TOT_B).transpose(1, 0, 2).reshape(QF, TOT_B))
    y_t = np.concatenate(parts, axis=0)
    out = np.ascontiguousarray(y_t.T).reshape(TOT_B, 1, DIM).astype(np.float32)
    return out, res


def kernel(**inputs):
    out, _ = _run(inputs, trace=False)
    return out
